# revision 13
# baseline (speedup 1.0000x reference)
"""Cross-attention Trainium2 kernel (Bass/Tile), data-parallel over batch.

B=8 batch elements -> 8 NeuronCores, one batch element per core.
Per core: y = softmax(q Wq (kv Wk)^T / sqrt(dk)) (kv Wv) Wo + bo
with S1=S2=2048, D=1024, H=8, DK=DV=128.

v3 design notes (all PE work is bf16, fp32 softmax stats in PSUM):
  - inputs cast f32->bf16 by SWDGE per 128-row chunk, transposed by the
    DMA xbar (dma_start_transpose); per-block layout xT[p, i, kc, 128]
    keeps each xbar destination contiguous.
  - SWDGE issue order matches consumption (Wk, kv j0, Wv, kv j1.., Wq,
    q j0.., Wo) so the first projection matmul can start ~15us in; a
    warmup matmul chain keeps the PE busy (and the HAM un-throttled)
    until real work arrives.
  - attention per (j,h): transposed scores ST = K_h^T q into 2-bank PSUM
    tiles, exp on ACT -> PT bf16; PV and row-sum matmuls interleaved per
    chunk (row sums use lhsT=ones[128,128], broadcasting sums to all
    partitions so the reciprocal runs full-width on DVE).
  - head loop software-pipelined one stage so PE does scores(h+1) while
    ACT exps (h); output projection of block j deferred to (j+1, h==1).
  - bias folded into the DVE PSUM->SBUF add of the output tiles.
"""

import os

import numpy as np

import concourse.bass as bass
import concourse.mybir as mybir
import concourse.tile as tile
from concourse import bacc
from concourse.bass_utils import run_bass_kernel_spmd

B = 8
S = 2048  # S1 == S2
D = 1024  # D1 == D2
H = 8
DK = DV = 128
KC = D // 128  # contraction chunks
SC = S // 128  # sequence chunks of 128
BLK = 512
NBLK = S // BLK
SCALE = 1.0 / float(np.sqrt(DK))
W_WARM = 64

F32 = mybir.dt.float32
BF16 = mybir.dt.bfloat16
EXP = mybir.ActivationFunctionType.Exp


def _emit(tc, aps):
    nc = tc.nc
    query, key_value, Wq, Wk, Wv, Wo, bo, out = (
        aps["query"], aps["key_value"], aps["Wq"], aps["Wk"], aps["Wv"],
        aps["Wo"], aps["bo"], aps["out"],
    )

    persist = tc.alloc_tile_pool(name="persist", bufs=1)
    QT_sb = persist.tile([128, H, S], BF16, name="QT_sb")
    KT_sb = persist.tile([128, H, S], BF16, name="KT_sb")
    V_sb = persist.tile([128, SC, H * DV], BF16, name="V_sb")
    Wo_sb = persist.tile([128, KC, D], BF16, name="Wo_sb")
    bo_bc = persist.tile([128, D], F32, name="bo_bc")
    ones_sb = persist.tile([128, 128], BF16, name="ones_sb")

    nc.vector.memset(ones_sb, 1.0)
    bo_bcast = bass.AP(
        tensor=bo.tensor, offset=bo.offset, ap=[[0, 128]] + list(bo.ap[1:])
    )
    nc.sync.dma_start(out=bo_bc, in_=bo_bcast)

    def load_weight(dst, src):
        # weights are bf16 in DRAM (host-cast); per-chunk HWDGE loads on
        # the ACT queue so they never queue behind the input transposes
        srcv = src.rearrange("(kc p) n -> p kc n", p=128)
        for kc in range(KC):
            nc.scalar.dma_start(out=dst[:, kc, :], in_=srcv[:, kc, :])

    # ---- phase 1: projections ----------------------------------------
    with nc.named_scope("ph1"), \
         tc.tile_pool(name="p1w", bufs=1) as wpool, \
         tc.tile_pool(name="p1work", bufs=1) as work, \
         tc.tile_pool(name="p1psum", bufs=4, space="PSUM") as pps, \
         tc.tile_pool(name="warmp", bufs=1, space="PSUM") as warmp:
        Wk_sb = wpool.tile([128, KC, D], BF16, name="Wk_sb")
        Wv_sb = wpool.tile([128, KC, D], BF16, name="Wv_sb")
        Wq_sb = wpool.tile([128, KC, D], BF16, name="Wq_sb")

        # warmup chain: keeps the PE issuing (and the HAM clock-gate
        # open) while the first weight/input DMAs land.
        wps = warmp.tile([128, 128], F32, name="wps")
        for w in range(W_WARM):
            nc.tensor.matmul(
                wps, lhsT=ones_sb, rhs=ones_sb,
                start=(w == 0), stop=(w == W_WARM - 1),
            )

        def transpose_block(src_ap, j, tag, bufs=3):
            """xbar-transpose rows [j*512, (j+1)*512) of src [S, D] (bf16,
            DRAM), one call per 128 rows so weight DMAs can interleave:
            xT[p, i, c, f] = src[j*512 + i*128 + f, c*128 + p]."""
            xT = work.tile([128, 4, KC, 128], BF16, name=f"{tag}T",
                           tag=f"{tag}T", bufs=bufs)
            for i in range(4):
                r0 = j * BLK + i * 128
                nc.sync.dma_start_transpose(
                    out=xT[:, i], in_=src_ap[r0:r0 + 128, :]
                )
            return xT

        def proj_headmajor(xT, W_sb, dst, j, tag):
            for m in range(H):
                ps = pps.tile([128, BLK], F32, name=f"ps_{tag}", tag="pps")
                for kc in range(KC):
                    nc.tensor.matmul(
                        ps, lhsT=W_sb[:, kc, m * 128:(m + 1) * 128],
                        rhs=xT[:, :, kc, :], start=(kc == 0), stop=(kc == KC - 1),
                    )
                if m % 2 == 0:
                    nc.scalar.copy(dst[:, m, j * BLK:(j + 1) * BLK], ps)
                else:
                    nc.vector.tensor_copy(dst[:, m, j * BLK:(j + 1) * BLK], ps)

        load_weight(Wk_sb, Wk)
        load_weight(Wv_sb, Wv)
        kvT_blocks = {j: transpose_block(key_value, j, "kv") for j in range(2)}
        load_weight(Wq_sb, Wq)
        load_weight(Wo_sb, Wo)
        for j in range(NBLK):
            kvT = kvT_blocks.pop(j)
            if j + 2 < NBLK:
                kvT_blocks[j + 2] = transpose_block(key_value, j + 2, "kv")
            proj_headmajor(kvT, Wk_sb, KT_sb, j, "k")
            for m4 in range(4):
                for n in range(2):
                    ps = pps.tile([128, BLK], F32, name="ps_v", tag="pps")
                    for kc in range(KC):
                        nc.tensor.matmul(
                            ps, lhsT=kvT[:, m4, kc, :],
                            rhs=Wv_sb[:, kc, n * BLK:(n + 1) * BLK],
                            start=(kc == 0), stop=(kc == KC - 1),
                        )
                    if n == 0:
                        nc.scalar.copy(
                            V_sb[:, j * 4 + m4, n * BLK:(n + 1) * BLK], ps
                        )
                    else:
                        nc.vector.tensor_copy(
                            V_sb[:, j * 4 + m4, n * BLK:(n + 1) * BLK], ps
                        )
        qT_blocks = {j: transpose_block(query, j, "q", bufs=2) for j in range(2)}
        for j in range(NBLK):
            qT = qT_blocks.pop(j)
            if j + 2 < NBLK:
                qT_blocks[j + 2] = transpose_block(query, j + 2, "q", bufs=2)
            proj_headmajor(qT, Wq_sb, QT_sb, j, "q")

    # ---- phase 2: attention + output projection ----------------------
    with nc.named_scope("attn"), \
         tc.tile_pool(name="p2", bufs=1) as p2, \
         tc.tile_pool(name="small", bufs=1) as small, \
         tc.tile_pool(name="spsum", bufs=2, space="PSUM") as spsum, \
         tc.tile_pool(name="opsum", bufs=1, space="PSUM") as opsum, \
         tc.tile_pool(name="rpsum", bufs=1, space="PSUM") as rpsum, \
         tc.tile_pool(name="ypsum", bufs=2, space="PSUM") as ypsum:

        OT_tiles = {}

        def finalize(j, h, ops, rps):
            rec = small.tile([128, BLK], F32, name="rec", tag="rec", bufs=2)
            nc.vector.reciprocal_approx_fast(out=rec, in_=rps)
            OT = OT_tiles[j]
            nc.vector.tensor_mul(OT[:, h, :], ops, rec)

        def head_iter(cur, prev):
            """Emit scores+exp for `cur` interleaved (in PE program order)
            with the PV / broadcast-row-sum matmuls of `prev`, so the PE
            has fill work while ACT exps drain.  Returns cur's PT."""
            j, h = cur
            PT = p2.tile([128, SC, BLK], BF16, name="PT", tag="PT", bufs=2)
            qblk = QT_sb[:, h, j * BLK:(j + 1) * BLK]
            if prev is not None:
                pj, ph, pPT = prev
                ops = opsum.tile([128, BLK], F32, name="ops", tag="ops")
                rps = rpsum.tile([128, BLK], F32, name="rps", tag="rps")
            for g in range(SC // 2):
                sps = spsum.tile([128, 2, BLK], F32, name="sps", tag="sps")
                for i in range(2):
                    c = 2 * g + i
                    nc.tensor.matmul(
                        sps[:, i, :],
                        lhsT=KT_sb[:, h, c * 128:(c + 1) * 128],
                        rhs=qblk, start=True, stop=True,
                    )
                if prev is not None:
                    for i in range(2):
                        c = 2 * g + i
                        nc.tensor.matmul(
                            ops, lhsT=V_sb[:, c, ph * 128:(ph + 1) * 128],
                            rhs=pPT[:, c, :], start=(c == 0), stop=(c == SC - 1),
                            skip_group_check=True,
                        )
                        nc.tensor.matmul(
                            rps, lhsT=ones_sb, rhs=pPT[:, c, :],
                            start=(c == 0), stop=(c == SC - 1),
                            skip_group_check=True,
                        )
                nc.scalar.activation(
                    PT[:, 2 * g:2 * (g + 1), :], sps, EXP, scale=SCALE
                )
            if prev is not None:
                finalize(pj, ph, ops, rps)
            return PT

        def pv_rowsum(j, h, PT):
            """Tail variant: PV + row-sums for the final head."""
            ops = opsum.tile([128, BLK], F32, name="ops", tag="ops")
            rps = rpsum.tile([128, BLK], F32, name="rps", tag="rps")
            for c in range(SC):
                nc.tensor.matmul(
                    ops, lhsT=V_sb[:, c, h * 128:(h + 1) * 128],
                    rhs=PT[:, c, :], start=(c == 0), stop=(c == SC - 1),
                    skip_group_check=True,
                )
                nc.tensor.matmul(
                    rps, lhsT=ones_sb, rhs=PT[:, c, :],
                    start=(c == 0), stop=(c == SC - 1),
                    skip_group_check=True,
                )
            finalize(j, h, ops, rps)

        def outproj(j):
            OT = OT_tiles[j]
            for m in range(4):
                for n in range(2):
                    yps = ypsum.tile([128, BLK], F32, name="yps", tag="yps")
                    for h in range(H):
                        nc.tensor.matmul(
                            yps, lhsT=OT[:, h, m * 128:(m + 1) * 128],
                            rhs=Wo_sb[:, h, n * BLK:(n + 1) * BLK],
                            start=(h == 0), stop=(h == H - 1),
                        )
                    y_sb = p2.tile([128, BLK], F32, name="y_sb", tag="y", bufs=3)
                    nc.vector.tensor_add(
                        y_sb, yps, bo_bc[:, n * BLK:(n + 1) * BLK]
                    )
                    r0 = j * BLK + m * 128
                    nc.sync.dma_start(
                        out=out[r0:r0 + 128, n * BLK:(n + 1) * BLK], in_=y_sb
                    )

        seq = [(j, h) for j in range(NBLK) for h in range(H)]
        prev = None
        for j, h in seq:
            if h == 0:
                OT_tiles[j] = p2.tile(
                    [128, H, BLK], BF16, name="OT", tag="OT", bufs=2
                )
            PT = head_iter((j, h), prev)
            if h == 1 and j > 0:
                outproj(j - 1)
            prev = (j, h, PT)
        pv_rowsum(prev[0], prev[1], prev[2])
        outproj(NBLK - 1)
    persist.release()


_CACHE = {}


def _build():
    if "nc" in _CACHE:
        return _CACHE["nc"]
    nc = bacc.Bacc(
        "TRN2", target_bir_lowering=False, debug=False,
        enable_asserts=False, num_devices=B,
    )
    aps = {
        "query": nc.dram_tensor("query", [S, D], BF16, kind="ExternalInput").ap(),
        "key_value": nc.dram_tensor("key_value", [S, D], BF16, kind="ExternalInput").ap(),
        "Wq": nc.dram_tensor("Wq", [D, H * DK], BF16, kind="ExternalInput").ap(),
        "Wk": nc.dram_tensor("Wk", [D, H * DK], BF16, kind="ExternalInput").ap(),
        "Wv": nc.dram_tensor("Wv", [D, H * DV], BF16, kind="ExternalInput").ap(),
        "Wo": nc.dram_tensor("Wo", [H * DV, D], BF16, kind="ExternalInput").ap(),
        "bo": nc.dram_tensor("bo", [1, D], F32, kind="ExternalInput").ap(),
        "out": nc.dram_tensor("out", [S, D], F32, kind="ExternalOutput").ap(),
    }
    with tile.TileContext(nc) as tc:
        _emit(tc, aps)
    nc.compile()
    _CACHE["nc"] = nc
    return nc


LAST_RESULT = None


def kernel(query, key_value, Wq, Wk, Wv, Wo, bo):
    global LAST_RESULT
    import ml_dtypes

    BF = ml_dtypes.bfloat16
    nc = _build()
    query = np.ascontiguousarray(np.asarray(query, dtype=np.float32).astype(BF))
    key_value = np.ascontiguousarray(
        np.asarray(key_value, dtype=np.float32).astype(BF)
    )
    shared = {
        "Wq": np.ascontiguousarray(np.asarray(Wq, dtype=np.float32).astype(BF)),
        "Wk": np.ascontiguousarray(np.asarray(Wk, dtype=np.float32).astype(BF)),
        "Wv": np.ascontiguousarray(np.asarray(Wv, dtype=np.float32).astype(BF)),
        "Wo": np.ascontiguousarray(np.asarray(Wo, dtype=np.float32).astype(BF)),
        "bo": np.ascontiguousarray(np.asarray(bo, dtype=np.float32)).reshape(1, D),
    }
    in_maps = [
        {"query": query[i], "key_value": key_value[i], **shared} for i in range(B)
    ]
    res = run_bass_kernel_spmd(
        nc, in_maps, core_ids=list(range(B)),
        trace=bool(int(os.environ.get("KERNEL_TRACE", "0"))),
    )
    LAST_RESULT = res
    return np.stack([r["out"] for r in res.results]).astype(np.float32)


if __name__ == "__main__":
    rng = np.random.default_rng(0)
    inputs = {
        "query": rng.standard_normal((B, S, D), dtype=np.float32),
        "key_value": rng.standard_normal((B, S, D), dtype=np.float32),
        "Wq": (rng.random((D, H * DK), dtype=np.float32) - 0.5) / 16.0,
        "Wk": (rng.random((D, H * DK), dtype=np.float32) - 0.5) / 16.0,
        "Wv": (rng.random((D, H * DV), dtype=np.float32) - 0.5) / 16.0,
        "Wo": (rng.random((H * DV, D), dtype=np.float32) - 0.5) / 16.0,
        "bo": (rng.random(D, dtype=np.float32) - 0.5) / 16.0,
    }
    y = kernel(**inputs)
    print("kernel out", y.shape, y.dtype, float(np.abs(y).max()))


# revision 18
# speedup vs baseline: 1.2569x; 1.2569x over previous
"""Cross-attention Trainium2 kernel (Bass/Tile), data-parallel over batch.

B=8 batch elements -> 8 NeuronCores, one batch element per core.
Per core: y = softmax(q Wq (kv Wk)^T / sqrt(dk)) (kv Wv) Wo + bo
with S1=S2=2048, D=1024, H=8, DK=DV=128.

v3 design notes (all PE work is bf16, fp32 softmax stats in PSUM):
  - inputs cast f32->bf16 by SWDGE per 128-row chunk, transposed by the
    DMA xbar (dma_start_transpose); per-block layout xT[p, i, kc, 128]
    keeps each xbar destination contiguous.
  - SWDGE issue order matches consumption (Wk, kv j0, Wv, kv j1.., Wq,
    q j0.., Wo) so the first projection matmul can start ~15us in; a
    warmup matmul chain keeps the PE busy (and the HAM un-throttled)
    until real work arrives.
  - attention per (j,h): transposed scores ST = K_h^T q into 2-bank PSUM
    tiles, exp on ACT -> PT bf16; PV and row-sum matmuls interleaved per
    chunk (row sums use lhsT=ones[128,128], broadcasting sums to all
    partitions so the reciprocal runs full-width on DVE).
  - head loop software-pipelined one stage so PE does scores(h+1) while
    ACT exps (h); output projection of block j deferred to (j+1, h==1).
  - bias folded into the DVE PSUM->SBUF add of the output tiles.
"""

import os

import numpy as np

import concourse.bass as bass
import concourse.mybir as mybir
import concourse.tile as tile
from concourse import bacc
from concourse.bass_utils import run_bass_kernel_spmd

B = 8
S = 2048  # S1 == S2
D = 1024  # D1 == D2
H = 8
DK = DV = 128
KC = D // 128  # contraction chunks
SC = S // 128  # sequence chunks of 128
BLK = 512
NBLK = S // BLK
SCALE = 1.0 / float(np.sqrt(DK))
W_WARM = 64

F32 = mybir.dt.float32
BF16 = mybir.dt.bfloat16
EXP = mybir.ActivationFunctionType.Exp


def _emit(tc, aps):
    nc = tc.nc
    query, key_value, Wq, Wk, Wv, Wo, bo, out = (
        aps["query"], aps["key_value"], aps["Wq"], aps["Wk"], aps["Wv"],
        aps["Wo"], aps["bo"], aps["out"],
    )

    persist = tc.alloc_tile_pool(name="persist", bufs=1)
    QT_sb = persist.tile([128, H, S], BF16, name="QT_sb")
    KT_sb = persist.tile([128, H, S], BF16, name="KT_sb")
    V_sb = persist.tile([128, SC, H * DV], BF16, name="V_sb")
    Wo_sb = persist.tile([128, KC, D], BF16, name="Wo_sb")
    bo_bc = persist.tile([128, D], F32, name="bo_bc")
    ones_sb = persist.tile([128, 128], BF16, name="ones_sb")

    nc.vector.memset(ones_sb, 1.0)
    bo_bcast = bass.AP(
        tensor=bo.tensor, offset=bo.offset, ap=[[0, 128]] + list(bo.ap[1:])
    )
    nc.sync.dma_start(out=bo_bc, in_=bo_bcast)

    def load_weight(dst, src):
        # weights are bf16 in DRAM (host-cast); per-chunk HWDGE loads on
        # the ACT queue so they never queue behind the input transposes
        srcv = src.rearrange("(kc p) n -> p kc n", p=128)
        for kc in range(KC):
            nc.scalar.dma_start(out=dst[:, kc, :], in_=srcv[:, kc, :])

    # ---- phase 1: projections ----------------------------------------
    with nc.named_scope("ph1"), \
         tc.tile_pool(name="p1w", bufs=1) as wpool, \
         tc.tile_pool(name="p1work", bufs=1) as work, \
         tc.tile_pool(name="p1psum", bufs=4, space="PSUM") as pps, \
         tc.tile_pool(name="warmp", bufs=1, space="PSUM") as warmp:
        Wk_sb = wpool.tile([128, KC, D], BF16, name="Wk_sb")
        Wv_sb = wpool.tile([128, KC, D], BF16, name="Wv_sb")
        Wq_sb = wpool.tile([128, KC, D], BF16, name="Wq_sb")

        # warmup chain: keeps the PE issuing (and the HAM clock-gate
        # open) while the first weight/input DMAs land.
        wps = warmp.tile([128, 128], F32, name="wps")
        for w in range(W_WARM):
            nc.tensor.matmul(
                wps, lhsT=ones_sb, rhs=ones_sb,
                start=(w == 0), stop=(w == W_WARM - 1),
            )

        def transpose_block(src_ap, j, tag, bufs=3):
            """xbar-transpose rows [j*512, (j+1)*512) of src [S, D] (bf16,
            DRAM) to xT[p, c, f] = src[j*512 + f, c*128 + p]."""
            xT = work.tile([128, KC, BLK], BF16, name=f"{tag}T",
                           tag=f"{tag}T", bufs=bufs)
            nc.sync.dma_start_transpose(
                out=xT, in_=src_ap[j * BLK:(j + 1) * BLK, :]
            )
            return xT

        def proj_headmajor(xT, W_sb, dst, j, tag):
            for m in range(H):
                ps = pps.tile([128, BLK], F32, name=f"ps_{tag}", tag="pps")
                for kc in range(KC):
                    nc.tensor.matmul(
                        ps, lhsT=W_sb[:, kc, m * 128:(m + 1) * 128],
                        rhs=xT[:, kc, :], start=(kc == 0), stop=(kc == KC - 1),
                    )
                if m % 2 == 0:
                    nc.scalar.copy(dst[:, m, j * BLK:(j + 1) * BLK], ps)
                else:
                    nc.vector.tensor_copy(dst[:, m, j * BLK:(j + 1) * BLK], ps)

        load_weight(Wk_sb, Wk)
        load_weight(Wv_sb, Wv)
        kvT_blocks = {j: transpose_block(key_value, j, "kv") for j in range(2)}
        load_weight(Wq_sb, Wq)
        load_weight(Wo_sb, Wo)
        for j in range(NBLK):
            kvT = kvT_blocks.pop(j)
            if j + 2 < NBLK:
                kvT_blocks[j + 2] = transpose_block(key_value, j + 2, "kv")
            proj_headmajor(kvT, Wk_sb, KT_sb, j, "k")
            for m4 in range(4):
                for n in range(2):
                    ps = pps.tile([128, BLK], F32, name="ps_v", tag="pps")
                    for kc in range(KC):
                        nc.tensor.matmul(
                            ps, lhsT=kvT[:, kc, m4 * 128:(m4 + 1) * 128],
                            rhs=Wv_sb[:, kc, n * BLK:(n + 1) * BLK],
                            start=(kc == 0), stop=(kc == KC - 1),
                        )
                    if n == 0:
                        nc.scalar.copy(
                            V_sb[:, j * 4 + m4, n * BLK:(n + 1) * BLK], ps
                        )
                    else:
                        nc.vector.tensor_copy(
                            V_sb[:, j * 4 + m4, n * BLK:(n + 1) * BLK], ps
                        )
        qT_blocks = {j: transpose_block(query, j, "q", bufs=2) for j in range(2)}
        for j in range(NBLK):
            qT = qT_blocks.pop(j)
            if j + 2 < NBLK:
                qT_blocks[j + 2] = transpose_block(query, j + 2, "q", bufs=2)
            proj_headmajor(qT, Wq_sb, QT_sb, j, "q")

    # ---- phase 2: attention + output projection ----------------------
    with nc.named_scope("attn"), \
         tc.tile_pool(name="p2", bufs=1) as p2, \
         tc.tile_pool(name="small", bufs=1) as small, \
         tc.tile_pool(name="spsum", bufs=2, space="PSUM") as spsum, \
         tc.tile_pool(name="opsum", bufs=1, space="PSUM") as opsum, \
         tc.tile_pool(name="rpsum", bufs=1, space="PSUM") as rpsum, \
         tc.tile_pool(name="ypsum", bufs=2, space="PSUM") as ypsum:

        OT_tiles = {}

        def finalize(j, h, PT, ops):
            """Row sums of PT via a DVE bf16 tree (in-place halving) + one
            broadcast matmul with lhsT=ones, then normalize into OT."""
            t8 = small.tile([128, 8, BLK], BF16, name="t8", tag="t8", bufs=2)
            nc.vector.tensor_add(t8, PT[:, 0:8, :], PT[:, 8:16, :])
            nc.vector.tensor_add(t8[:, 0:4], t8[:, 0:4], t8[:, 4:8])
            nc.vector.tensor_add(t8[:, 0:2], t8[:, 0:2], t8[:, 2:4])
            nc.vector.tensor_add(t8[:, 0, :], t8[:, 0, :], t8[:, 1, :])
            rps = rpsum.tile([128, BLK], F32, name="rps", tag="rps")
            nc.tensor.matmul(
                rps, lhsT=ones_sb, rhs=t8[:, 0, :], start=True, stop=True
            )
            rec = small.tile([128, BLK], F32, name="rec", tag="rec", bufs=2)
            nc.vector.reciprocal_approx_fast(out=rec, in_=rps)
            OT = OT_tiles[j]
            nc.vector.tensor_mul(OT[:, h, :], ops, rec)

        def head_iter(cur, prev):
            """Emit scores+exp for `cur` interleaved (in PE program order)
            with the PV matmuls of `prev`, so the PE has fill work while
            ACT exps drain.  Returns cur's PT."""
            j, h = cur
            PT = p2.tile([128, SC, BLK], BF16, name="PT", tag="PT", bufs=2)
            qblk = QT_sb[:, h, j * BLK:(j + 1) * BLK]
            if prev is not None:
                pj, ph, pPT = prev
                ops = opsum.tile([128, BLK], F32, name="ops", tag="ops")
            for g in range(SC // 2):
                sps = spsum.tile([128, 2, BLK], F32, name="sps", tag="sps")
                for i in range(2):
                    c = 2 * g + i
                    nc.tensor.matmul(
                        sps[:, i, :],
                        lhsT=KT_sb[:, h, c * 128:(c + 1) * 128],
                        rhs=qblk, start=True, stop=True,
                    )
                if prev is not None:
                    for i in range(2):
                        c = 2 * g + i
                        nc.tensor.matmul(
                            ops, lhsT=V_sb[:, c, ph * 128:(ph + 1) * 128],
                            rhs=pPT[:, c, :], start=(c == 0), stop=(c == SC - 1),
                            skip_group_check=True,
                        )
                nc.scalar.activation(
                    PT[:, 2 * g:2 * (g + 1), :], sps, EXP, scale=SCALE
                )
            if prev is not None:
                finalize(pj, ph, pPT, ops)
            return PT

        def pv_rowsum(j, h, PT):
            """Tail variant: PV + row sums for the final head."""
            ops = opsum.tile([128, BLK], F32, name="ops", tag="ops")
            for c in range(SC):
                nc.tensor.matmul(
                    ops, lhsT=V_sb[:, c, h * 128:(h + 1) * 128],
                    rhs=PT[:, c, :], start=(c == 0), stop=(c == SC - 1),
                    skip_group_check=True,
                )
            finalize(j, h, PT, ops)

        def outproj(j):
            OT = OT_tiles[j]
            for m in range(4):
                for n in range(2):
                    yps = ypsum.tile([128, BLK], F32, name="yps", tag="yps")
                    for h in range(H):
                        nc.tensor.matmul(
                            yps, lhsT=OT[:, h, m * 128:(m + 1) * 128],
                            rhs=Wo_sb[:, h, n * BLK:(n + 1) * BLK],
                            start=(h == 0), stop=(h == H - 1),
                        )
                    y_sb = p2.tile([128, BLK], F32, name="y_sb", tag="y", bufs=3)
                    nc.vector.tensor_add(
                        y_sb, yps, bo_bc[:, n * BLK:(n + 1) * BLK]
                    )
                    r0 = j * BLK + m * 128
                    nc.sync.dma_start(
                        out=out[r0:r0 + 128, n * BLK:(n + 1) * BLK], in_=y_sb
                    )

        seq = [(j, h) for j in range(NBLK) for h in range(H)]
        prev = None
        for j, h in seq:
            if h == 0:
                OT_tiles[j] = p2.tile(
                    [128, H, BLK], BF16, name="OT", tag="OT", bufs=2
                )
            PT = head_iter((j, h), prev)
            if h == 1 and j > 0:
                outproj(j - 1)
            prev = (j, h, PT)
        pv_rowsum(prev[0], prev[1], prev[2])
        outproj(NBLK - 1)
    persist.release()


_CACHE = {}


def _build():
    if "nc" in _CACHE:
        return _CACHE["nc"]
    nc = bacc.Bacc(
        "TRN2", target_bir_lowering=False, debug=False,
        enable_asserts=False, num_devices=B,
    )
    aps = {
        "query": nc.dram_tensor("query", [S, D], BF16, kind="ExternalInput").ap(),
        "key_value": nc.dram_tensor("key_value", [S, D], BF16, kind="ExternalInput").ap(),
        "Wq": nc.dram_tensor("Wq", [D, H * DK], BF16, kind="ExternalInput").ap(),
        "Wk": nc.dram_tensor("Wk", [D, H * DK], BF16, kind="ExternalInput").ap(),
        "Wv": nc.dram_tensor("Wv", [D, H * DV], BF16, kind="ExternalInput").ap(),
        "Wo": nc.dram_tensor("Wo", [H * DV, D], BF16, kind="ExternalInput").ap(),
        "bo": nc.dram_tensor("bo", [1, D], F32, kind="ExternalInput").ap(),
        "out": nc.dram_tensor("out", [S, D], F32, kind="ExternalOutput").ap(),
    }
    with tile.TileContext(nc) as tc:
        _emit(tc, aps)
    nc.compile()
    _CACHE["nc"] = nc
    return nc


LAST_RESULT = None


def kernel(query, key_value, Wq, Wk, Wv, Wo, bo):
    global LAST_RESULT
    import ml_dtypes

    BF = ml_dtypes.bfloat16
    nc = _build()
    query = np.ascontiguousarray(np.asarray(query, dtype=np.float32).astype(BF))
    key_value = np.ascontiguousarray(
        np.asarray(key_value, dtype=np.float32).astype(BF)
    )
    shared = {
        "Wq": np.ascontiguousarray(np.asarray(Wq, dtype=np.float32).astype(BF)),
        "Wk": np.ascontiguousarray(np.asarray(Wk, dtype=np.float32).astype(BF)),
        "Wv": np.ascontiguousarray(np.asarray(Wv, dtype=np.float32).astype(BF)),
        "Wo": np.ascontiguousarray(np.asarray(Wo, dtype=np.float32).astype(BF)),
        "bo": np.ascontiguousarray(np.asarray(bo, dtype=np.float32)).reshape(1, D),
    }
    in_maps = [
        {"query": query[i], "key_value": key_value[i], **shared} for i in range(B)
    ]
    res = run_bass_kernel_spmd(
        nc, in_maps, core_ids=list(range(B)),
        trace=bool(int(os.environ.get("KERNEL_TRACE", "0"))),
    )
    LAST_RESULT = res
    return np.stack([r["out"] for r in res.results]).astype(np.float32)


if __name__ == "__main__":
    rng = np.random.default_rng(0)
    inputs = {
        "query": rng.standard_normal((B, S, D), dtype=np.float32),
        "key_value": rng.standard_normal((B, S, D), dtype=np.float32),
        "Wq": (rng.random((D, H * DK), dtype=np.float32) - 0.5) / 16.0,
        "Wk": (rng.random((D, H * DK), dtype=np.float32) - 0.5) / 16.0,
        "Wv": (rng.random((D, H * DV), dtype=np.float32) - 0.5) / 16.0,
        "Wo": (rng.random((H * DV, D), dtype=np.float32) - 0.5) / 16.0,
        "bo": (rng.random(D, dtype=np.float32) - 0.5) / 16.0,
    }
    y = kernel(**inputs)
    print("kernel out", y.shape, y.dtype, float(np.abs(y).max()))


# revision 21
# speedup vs baseline: 1.3185x; 1.0491x over previous
"""Cross-attention Trainium2 kernel (Bass/Tile), data-parallel over batch.

B=8 batch elements -> 8 NeuronCores, one batch element per core.
Per core: y = softmax(q Wq (kv Wk)^T / sqrt(dk)) (kv Wv) Wo + bo
with S1=S2=2048, D=1024, H=8, DK=DV=128.

v3 design notes (all PE work is bf16, fp32 softmax stats in PSUM):
  - inputs cast f32->bf16 by SWDGE per 128-row chunk, transposed by the
    DMA xbar (dma_start_transpose); per-block layout xT[p, i, kc, 128]
    keeps each xbar destination contiguous.
  - SWDGE issue order matches consumption (Wk, kv j0, Wv, kv j1.., Wq,
    q j0.., Wo) so the first projection matmul can start ~15us in; a
    warmup matmul chain keeps the PE busy (and the HAM un-throttled)
    until real work arrives.
  - attention per (j,h): transposed scores ST = K_h^T q into 2-bank PSUM
    tiles, exp on ACT -> PT bf16; PV and row-sum matmuls interleaved per
    chunk (row sums use lhsT=ones[128,128], broadcasting sums to all
    partitions so the reciprocal runs full-width on DVE).
  - head loop software-pipelined one stage so PE does scores(h+1) while
    ACT exps (h); output projection of block j deferred to (j+1, h==1).
  - bias folded into the DVE PSUM->SBUF add of the output tiles.
"""

import os

import numpy as np

import concourse.bass as bass
import concourse.mybir as mybir
import concourse.tile as tile
from concourse import bacc
from concourse.bass_utils import run_bass_kernel_spmd

B = 8
S = 2048  # S1 == S2
D = 1024  # D1 == D2
H = 8
DK = DV = 128
KC = D // 128  # contraction chunks
SC = S // 128  # sequence chunks of 128
BLK = 512
NBLK = S // BLK
SCALE = 1.0 / float(np.sqrt(DK))
W_WARM = 100

F32 = mybir.dt.float32
BF16 = mybir.dt.bfloat16
EXP = mybir.ActivationFunctionType.Exp


def _emit(tc, aps):
    nc = tc.nc
    query, key_value, Wq, Wk, Wv, Wo, bo, out = (
        aps["query"], aps["key_value"], aps["Wq"], aps["Wk"], aps["Wv"],
        aps["Wo"], aps["bo"], aps["out"],
    )

    persist = tc.alloc_tile_pool(name="persist", bufs=1)
    QT_sb = persist.tile([128, H, S], BF16, name="QT_sb")
    KT_sb = persist.tile([128, H, S], BF16, name="KT_sb")
    V_sb = persist.tile([128, SC, H * DV], BF16, name="V_sb")
    Wo_sb = persist.tile([128, KC, D], BF16, name="Wo_sb")
    bo_bc = persist.tile([128, D], F32, name="bo_bc")
    ones_sb = persist.tile([128, 128], BF16, name="ones_sb")

    nc.vector.memset(ones_sb, 1.0)

    def load_weight(dst, src):
        # weights are bf16 in DRAM (host-cast); one HWDGE DMA per weight
        # on the ACT queue (splitting into chunks burns semaphore slots,
        # which serialize the whole load stream against compute)
        nc.scalar.dma_start(out=dst, in_=src.rearrange("(kc p) n -> p kc n", p=128))

    # ---- phase 1: projections ----------------------------------------
    with nc.named_scope("ph1"), \
         tc.tile_pool(name="p1w", bufs=1) as wpool, \
         tc.tile_pool(name="p1work", bufs=1) as work, \
         tc.tile_pool(name="p1psum", bufs=4, space="PSUM") as pps, \
         tc.tile_pool(name="warmp", bufs=1, space="PSUM") as warmp:
        Wk_sb = wpool.tile([128, KC, D], BF16, name="Wk_sb")
        Wv_sb = wpool.tile([128, KC, D], BF16, name="Wv_sb")
        Wq_sb = wpool.tile([128, KC, D], BF16, name="Wq_sb")

        # warmup chain: keeps the PE issuing (and the HAM clock-gate
        # open) while the first weight/input DMAs land.
        wps = warmp.tile([128, 128], F32, name="wps")
        for w in range(W_WARM):
            nc.tensor.matmul(
                wps, lhsT=ones_sb, rhs=ones_sb,
                start=(w == 0), stop=(w == W_WARM - 1),
            )

        def transpose_block(src_ap, j, tag, bufs=3):
            """xbar-transpose rows [j*512, (j+1)*512) of src [S, D] (bf16,
            DRAM) to xT[p, c, f] = src[j*512 + f, c*128 + p]."""
            xT = work.tile([128, KC, BLK], BF16, name=f"{tag}T",
                           tag=f"{tag}T", bufs=bufs)
            nc.sync.dma_start_transpose(
                out=xT, in_=src_ap[j * BLK:(j + 1) * BLK, :]
            )
            return xT

        def proj_headmajor(xT, W_sb, dst, j, tag):
            for m in range(H):
                ps = pps.tile([128, BLK], F32, name=f"ps_{tag}", tag="pps")
                for kc in range(KC):
                    nc.tensor.matmul(
                        ps, lhsT=W_sb[:, kc, m * 128:(m + 1) * 128],
                        rhs=xT[:, kc, :], start=(kc == 0), stop=(kc == KC - 1),
                    )
                if m % 2 == 0:
                    nc.scalar.copy(dst[:, m, j * BLK:(j + 1) * BLK], ps)
                else:
                    nc.vector.tensor_copy(dst[:, m, j * BLK:(j + 1) * BLK], ps)

        load_weight(Wk_sb, Wk)
        load_weight(Wv_sb, Wv)
        kvT_blocks = {j: transpose_block(key_value, j, "kv") for j in range(2)}
        load_weight(Wq_sb, Wq)
        load_weight(Wo_sb, Wo)
        for j in range(NBLK):
            kvT = kvT_blocks.pop(j)
            if j + 2 < NBLK:
                kvT_blocks[j + 2] = transpose_block(key_value, j + 2, "kv")
            proj_headmajor(kvT, Wk_sb, KT_sb, j, "k")
            for m4 in range(4):
                for n in range(2):
                    ps = pps.tile([128, BLK], F32, name="ps_v", tag="pps")
                    for kc in range(KC):
                        nc.tensor.matmul(
                            ps, lhsT=kvT[:, kc, m4 * 128:(m4 + 1) * 128],
                            rhs=Wv_sb[:, kc, n * BLK:(n + 1) * BLK],
                            start=(kc == 0), stop=(kc == KC - 1),
                        )
                    if n == 0:
                        nc.scalar.copy(
                            V_sb[:, j * 4 + m4, n * BLK:(n + 1) * BLK], ps
                        )
                    else:
                        nc.vector.tensor_copy(
                            V_sb[:, j * 4 + m4, n * BLK:(n + 1) * BLK], ps
                        )
        qT_blocks = {j: transpose_block(query, j, "q", bufs=2) for j in range(2)}
        bo_bcast = bass.AP(
            tensor=bo.tensor, offset=bo.offset, ap=[[0, 128]] + list(bo.ap[1:])
        )
        nc.sync.dma_start(out=bo_bc, in_=bo_bcast)
        for j in range(NBLK):
            qT = qT_blocks.pop(j)
            if j + 2 < NBLK:
                qT_blocks[j + 2] = transpose_block(query, j + 2, "q", bufs=2)
            proj_headmajor(qT, Wq_sb, QT_sb, j, "q")

    # ---- phase 2: attention + output projection ----------------------
    with nc.named_scope("attn"), \
         tc.tile_pool(name="p2", bufs=1) as p2, \
         tc.tile_pool(name="small", bufs=1) as small, \
         tc.tile_pool(name="spsum", bufs=2, space="PSUM") as spsum, \
         tc.tile_pool(name="opsum", bufs=1, space="PSUM") as opsum, \
         tc.tile_pool(name="rpsum", bufs=1, space="PSUM") as rpsum, \
         tc.tile_pool(name="ypsum", bufs=2, space="PSUM") as ypsum:

        OT_tiles = {}

        def finalize(j, h, PT, ops):
            """Row sums of PT via a DVE bf16 tree (in-place halving) + one
            broadcast matmul with lhsT=ones, then normalize into OT."""
            t8 = small.tile([128, 8, BLK], BF16, name="t8", tag="t8", bufs=2)
            nc.vector.tensor_add(t8, PT[:, 0:8, :], PT[:, 8:16, :])
            nc.vector.tensor_add(t8[:, 0:4], t8[:, 0:4], t8[:, 4:8])
            nc.vector.tensor_add(t8[:, 0:2], t8[:, 0:2], t8[:, 2:4])
            nc.vector.tensor_add(t8[:, 0, :], t8[:, 0, :], t8[:, 1, :])
            rps = rpsum.tile([128, BLK], F32, name="rps", tag="rps")
            nc.tensor.matmul(
                rps, lhsT=ones_sb, rhs=t8[:, 0, :], start=True, stop=True
            )
            rec = small.tile([128, BLK], F32, name="rec", tag="rec", bufs=2)
            nc.vector.reciprocal_approx_fast(out=rec, in_=rps)
            OT = OT_tiles[j]
            nc.vector.tensor_mul(OT[:, h, :], ops, rec)

        def head_iter(cur, prev):
            """Emit scores+exp for `cur` interleaved (in PE program order)
            with the PV matmuls of `prev`, so the PE has fill work while
            ACT exps drain.  Returns cur's PT."""
            j, h = cur
            PT = p2.tile([128, SC, BLK], BF16, name="PT", tag="PT", bufs=2)
            qblk = QT_sb[:, h, j * BLK:(j + 1) * BLK]
            if prev is not None:
                pj, ph, pPT = prev
                ops = opsum.tile([128, BLK], F32, name="ops", tag="ops")
            for g in range(SC // 2):
                sps = spsum.tile([128, 2, BLK], F32, name="sps", tag="sps")
                for i in range(2):
                    c = 2 * g + i
                    nc.tensor.matmul(
                        sps[:, i, :],
                        lhsT=KT_sb[:, h, c * 128:(c + 1) * 128],
                        rhs=qblk, start=True, stop=True,
                    )
                if prev is not None:
                    for i in range(2):
                        c = 2 * g + i
                        nc.tensor.matmul(
                            ops, lhsT=V_sb[:, c, ph * 128:(ph + 1) * 128],
                            rhs=pPT[:, c, :], start=(c == 0), stop=(c == SC - 1),
                            skip_group_check=True,
                        )
                nc.scalar.activation(
                    PT[:, 2 * g:2 * (g + 1), :], sps, EXP, scale=SCALE
                )
            if prev is not None:
                finalize(pj, ph, pPT, ops)
            return PT

        def pv_rowsum(j, h, PT):
            """Tail variant: PV + row sums for the final head."""
            ops = opsum.tile([128, BLK], F32, name="ops", tag="ops")
            for c in range(SC):
                nc.tensor.matmul(
                    ops, lhsT=V_sb[:, c, h * 128:(h + 1) * 128],
                    rhs=PT[:, c, :], start=(c == 0), stop=(c == SC - 1),
                    skip_group_check=True,
                )
            finalize(j, h, PT, ops)

        def outproj(j):
            OT = OT_tiles[j]
            for m in range(4):
                for n in range(2):
                    yps = ypsum.tile([128, BLK], F32, name="yps", tag="yps")
                    for h in range(H):
                        nc.tensor.matmul(
                            yps, lhsT=OT[:, h, m * 128:(m + 1) * 128],
                            rhs=Wo_sb[:, h, n * BLK:(n + 1) * BLK],
                            start=(h == 0), stop=(h == H - 1),
                        )
                    y_sb = p2.tile([128, BLK], F32, name="y_sb", tag="y", bufs=3)
                    nc.vector.tensor_add(
                        y_sb, yps, bo_bc[:, n * BLK:(n + 1) * BLK]
                    )
                    r0 = j * BLK + m * 128
                    nc.sync.dma_start(
                        out=out[r0:r0 + 128, n * BLK:(n + 1) * BLK], in_=y_sb
                    )

        seq = [(j, h) for j in range(NBLK) for h in range(H)]
        prev = None
        for j, h in seq:
            if h == 0:
                OT_tiles[j] = p2.tile(
                    [128, H, BLK], BF16, name="OT", tag="OT", bufs=2
                )
            PT = head_iter((j, h), prev)
            if h == 1 and j > 0:
                outproj(j - 1)
            prev = (j, h, PT)
        pv_rowsum(prev[0], prev[1], prev[2])
        outproj(NBLK - 1)
    persist.release()


_CACHE = {}


def _build():
    if "nc" in _CACHE:
        return _CACHE["nc"]
    nc = bacc.Bacc(
        "TRN2", target_bir_lowering=False, debug=False,
        enable_asserts=False, num_devices=B,
    )
    aps = {
        "query": nc.dram_tensor("query", [S, D], BF16, kind="ExternalInput").ap(),
        "key_value": nc.dram_tensor("key_value", [S, D], BF16, kind="ExternalInput").ap(),
        "Wq": nc.dram_tensor("Wq", [D, H * DK], BF16, kind="ExternalInput").ap(),
        "Wk": nc.dram_tensor("Wk", [D, H * DK], BF16, kind="ExternalInput").ap(),
        "Wv": nc.dram_tensor("Wv", [D, H * DV], BF16, kind="ExternalInput").ap(),
        "Wo": nc.dram_tensor("Wo", [H * DV, D], BF16, kind="ExternalInput").ap(),
        "bo": nc.dram_tensor("bo", [1, D], F32, kind="ExternalInput").ap(),
        "out": nc.dram_tensor("out", [S, D], F32, kind="ExternalOutput").ap(),
    }
    with tile.TileContext(nc) as tc:
        _emit(tc, aps)
    nc.compile()
    _CACHE["nc"] = nc
    return nc


LAST_RESULT = None


def kernel(query, key_value, Wq, Wk, Wv, Wo, bo):
    global LAST_RESULT
    import ml_dtypes

    BF = ml_dtypes.bfloat16
    nc = _build()
    query = np.ascontiguousarray(np.asarray(query, dtype=np.float32).astype(BF))
    key_value = np.ascontiguousarray(
        np.asarray(key_value, dtype=np.float32).astype(BF)
    )
    shared = {
        "Wq": np.ascontiguousarray(np.asarray(Wq, dtype=np.float32).astype(BF)),
        "Wk": np.ascontiguousarray(np.asarray(Wk, dtype=np.float32).astype(BF)),
        "Wv": np.ascontiguousarray(np.asarray(Wv, dtype=np.float32).astype(BF)),
        "Wo": np.ascontiguousarray(np.asarray(Wo, dtype=np.float32).astype(BF)),
        "bo": np.ascontiguousarray(np.asarray(bo, dtype=np.float32)).reshape(1, D),
    }
    in_maps = [
        {"query": query[i], "key_value": key_value[i], **shared} for i in range(B)
    ]
    res = run_bass_kernel_spmd(
        nc, in_maps, core_ids=list(range(B)),
        trace=bool(int(os.environ.get("KERNEL_TRACE", "0"))),
    )
    LAST_RESULT = res
    return np.stack([r["out"] for r in res.results]).astype(np.float32)


if __name__ == "__main__":
    rng = np.random.default_rng(0)
    inputs = {
        "query": rng.standard_normal((B, S, D), dtype=np.float32),
        "key_value": rng.standard_normal((B, S, D), dtype=np.float32),
        "Wq": (rng.random((D, H * DK), dtype=np.float32) - 0.5) / 16.0,
        "Wk": (rng.random((D, H * DK), dtype=np.float32) - 0.5) / 16.0,
        "Wv": (rng.random((D, H * DV), dtype=np.float32) - 0.5) / 16.0,
        "Wo": (rng.random((H * DV, D), dtype=np.float32) - 0.5) / 16.0,
        "bo": (rng.random(D, dtype=np.float32) - 0.5) / 16.0,
    }
    y = kernel(**inputs)
    print("kernel out", y.shape, y.dtype, float(np.abs(y).max()))


# revision 25
# speedup vs baseline: 1.3621x; 1.0330x over previous
"""Cross-attention Trainium2 kernel (Bass/Tile), data-parallel over batch.

B=8 batch elements -> 8 NeuronCores, one batch element per core.
Per core: y = softmax(q Wq (kv Wk)^T / sqrt(dk)) (kv Wv) Wo + bo
with S1=S2=2048, D=1024, H=8, DK=DV=128.

v3 design notes (all PE work is bf16, fp32 softmax stats in PSUM):
  - inputs cast f32->bf16 by SWDGE per 128-row chunk, transposed by the
    DMA xbar (dma_start_transpose); per-block layout xT[p, i, kc, 128]
    keeps each xbar destination contiguous.
  - SWDGE issue order matches consumption (Wk, kv j0, Wv, kv j1.., Wq,
    q j0.., Wo) so the first projection matmul can start ~15us in; a
    warmup matmul chain keeps the PE busy (and the HAM un-throttled)
    until real work arrives.
  - attention per (j,h): transposed scores ST = K_h^T q into 2-bank PSUM
    tiles, exp on ACT -> PT bf16; PV and row-sum matmuls interleaved per
    chunk (row sums use lhsT=ones[128,128], broadcasting sums to all
    partitions so the reciprocal runs full-width on DVE).
  - head loop software-pipelined one stage so PE does scores(h+1) while
    ACT exps (h); output projection of block j deferred to (j+1, h==1).
  - bias folded into the DVE PSUM->SBUF add of the output tiles.
"""

import os

import numpy as np

import concourse.bass as bass
import concourse.mybir as mybir
import concourse.tile as tile
from concourse import bacc
from concourse.bass_utils import run_bass_kernel_spmd

B = 8
S = 2048  # S1 == S2
D = 1024  # D1 == D2
H = 8
DK = DV = 128
KC = D // 128  # contraction chunks
SC = S // 128  # sequence chunks of 128
BLK = 512
NBLK = S // BLK
SCALE = 1.0 / float(np.sqrt(DK))
W_WARM = 100

F32 = mybir.dt.float32
BF16 = mybir.dt.bfloat16
EXP = mybir.ActivationFunctionType.Exp


def _emit(tc, aps):
    nc = tc.nc
    query, key_value, Wq, Wk, Wv, Wo, bo, out = (
        aps["query"], aps["key_value"], aps["Wq"], aps["Wk"], aps["Wv"],
        aps["Wo"], aps["bo"], aps["out"],
    )

    persist = tc.alloc_tile_pool(name="persist", bufs=1)
    QT_sb = persist.tile([128, H, S], BF16, name="QT_sb")
    KT_sb = persist.tile([128, H, S], BF16, name="KT_sb")
    V_sb = persist.tile([128, SC, H * DV], BF16, name="V_sb")
    Wo_sb = persist.tile([128, KC, D], BF16, name="Wo_sb")
    bo_bc = persist.tile([128, D], F32, name="bo_bc")
    ones_sb = persist.tile([128, 128], BF16, name="ones_sb")

    nc.vector.memset(ones_sb, 1.0)

    def load_weight(dst, src):
        # weights are bf16 in DRAM (host-cast); one HWDGE DMA per weight,
        # on the same queue as the input transposes so the whole load
        # stream executes in exact consumption order (cross-queue DMAs
        # interleave unpredictably on the shared engines)
        nc.sync.dma_start(out=dst, in_=src.rearrange("(kc p) n -> p kc n", p=128))

    # ---- phase 1: projections ----------------------------------------
    with nc.named_scope("ph1"), \
         tc.tile_pool(name="p1w", bufs=1) as wpool, \
         tc.tile_pool(name="p1work", bufs=1) as work, \
         tc.tile_pool(name="p1psum", bufs=4, space="PSUM") as pps, \
         tc.tile_pool(name="warmp", bufs=1, space="PSUM") as warmp:
        Wk_sb = wpool.tile([128, KC, D], BF16, name="Wk_sb")
        Wv_sb = wpool.tile([128, KC, D], BF16, name="Wv_sb")
        Wq_sb = wpool.tile([128, KC, D], BF16, name="Wq_sb")

        # warmup chain: keeps the PE issuing (and the HAM clock-gate
        # open) while the first weight/input DMAs land.
        wps = warmp.tile([128, 128], F32, name="wps")
        for w in range(W_WARM):
            nc.tensor.matmul(
                wps, lhsT=ones_sb, rhs=ones_sb,
                start=(w == 0), stop=(w == W_WARM - 1),
            )

        def transpose_block(src_ap, j, tag, bufs=3):
            """xbar-transpose rows [j*512, (j+1)*512) of src [S, D] (bf16,
            DRAM) to xT[p, c, f] = src[j*512 + f, c*128 + p]."""
            xT = work.tile([128, KC, BLK], BF16, name=f"{tag}T",
                           tag=f"{tag}T", bufs=bufs)
            nc.sync.dma_start_transpose(
                out=xT, in_=src_ap[j * BLK:(j + 1) * BLK, :]
            )
            return xT

        def proj_headmajor(xT, W_sb, dst, j, tag):
            for m in range(H):
                ps = pps.tile([128, BLK], F32, name=f"ps_{tag}", tag="pps")
                for kc in range(KC):
                    nc.tensor.matmul(
                        ps, lhsT=W_sb[:, kc, m * 128:(m + 1) * 128],
                        rhs=xT[:, kc, :], start=(kc == 0), stop=(kc == KC - 1),
                    )
                if m % 2 == 0:
                    nc.scalar.copy(dst[:, m, j * BLK:(j + 1) * BLK], ps)
                else:
                    nc.vector.tensor_copy(dst[:, m, j * BLK:(j + 1) * BLK], ps)

        load_weight(Wk_sb, Wk)
        kvT_blocks = {0: transpose_block(key_value, 0, "kv")}
        load_weight(Wv_sb, Wv)
        kvT_blocks[1] = transpose_block(key_value, 1, "kv")
        for j in range(NBLK):
            kvT = kvT_blocks.pop(j)
            if j + 2 < NBLK:
                kvT_blocks[j + 2] = transpose_block(key_value, j + 2, "kv")
            elif j == 2:
                load_weight(Wq_sb, Wq)
            elif j == 3:
                load_weight(Wo_sb, Wo)
            proj_headmajor(kvT, Wk_sb, KT_sb, j, "k")
            for m4 in range(4):
                for n in range(2):
                    ps = pps.tile([128, BLK], F32, name="ps_v", tag="pps")
                    for kc in range(KC):
                        nc.tensor.matmul(
                            ps, lhsT=kvT[:, kc, m4 * 128:(m4 + 1) * 128],
                            rhs=Wv_sb[:, kc, n * BLK:(n + 1) * BLK],
                            start=(kc == 0), stop=(kc == KC - 1),
                        )
                    if n == 0:
                        nc.scalar.copy(
                            V_sb[:, j * 4 + m4, n * BLK:(n + 1) * BLK], ps
                        )
                    else:
                        nc.vector.tensor_copy(
                            V_sb[:, j * 4 + m4, n * BLK:(n + 1) * BLK], ps
                        )
        qT_blocks = {j: transpose_block(query, j, "q", bufs=2) for j in range(2)}
        bo_bcast = bass.AP(
            tensor=bo.tensor, offset=bo.offset, ap=[[0, 128]] + list(bo.ap[1:])
        )
        nc.sync.dma_start(out=bo_bc, in_=bo_bcast)
        for j in range(NBLK):
            qT = qT_blocks.pop(j)
            if j + 2 < NBLK:
                qT_blocks[j + 2] = transpose_block(query, j + 2, "q", bufs=2)
            proj_headmajor(qT, Wq_sb, QT_sb, j, "q")

    # ---- phase 2: attention + output projection ----------------------
    with nc.named_scope("attn"), \
         tc.tile_pool(name="p2", bufs=1) as p2, \
         tc.tile_pool(name="small", bufs=1) as small, \
         tc.tile_pool(name="spsum", bufs=2, space="PSUM") as spsum, \
         tc.tile_pool(name="opsum", bufs=1, space="PSUM") as opsum, \
         tc.tile_pool(name="rpsum", bufs=1, space="PSUM") as rpsum, \
         tc.tile_pool(name="ypsum", bufs=2, space="PSUM") as ypsum:

        OT_tiles = {}

        def finalize(j, h, PT, ops):
            """Row sums of PT via a DVE bf16 tree (in-place halving) + one
            broadcast matmul with lhsT=ones, then normalize into OT."""
            t8 = small.tile([128, 8, BLK], BF16, name="t8", tag="t8", bufs=2)
            nc.vector.tensor_add(t8, PT[:, 0:8, :], PT[:, 8:16, :])
            nc.vector.tensor_add(t8[:, 0:4], t8[:, 0:4], t8[:, 4:8])
            nc.vector.tensor_add(t8[:, 0:2], t8[:, 0:2], t8[:, 2:4])
            nc.vector.tensor_add(t8[:, 0, :], t8[:, 0, :], t8[:, 1, :])
            rps = rpsum.tile([128, BLK], F32, name="rps", tag="rps")
            nc.tensor.matmul(
                rps, lhsT=ones_sb, rhs=t8[:, 0, :], start=True, stop=True
            )
            rec = small.tile([128, BLK], F32, name="rec", tag="rec", bufs=2)
            nc.vector.reciprocal_approx_fast(out=rec, in_=rps)
            OT = OT_tiles[j]
            nc.vector.tensor_mul(OT[:, h, :], ops, rec)

        def head_iter(cur, prev):
            """Emit scores+exp for `cur` interleaved (in PE program order)
            with the PV matmuls of `prev`, so the PE has fill work while
            ACT exps drain.  Returns cur's PT."""
            j, h = cur
            PT = p2.tile([128, SC, BLK], BF16, name="PT", tag="PT", bufs=2)
            qblk = QT_sb[:, h, j * BLK:(j + 1) * BLK]
            if prev is not None:
                pj, ph, pPT = prev
                ops = opsum.tile([128, BLK], F32, name="ops", tag="ops")
            for g in range(SC // 2):
                sps = spsum.tile([128, 2, BLK], F32, name="sps", tag="sps")
                for i in range(2):
                    c = 2 * g + i
                    nc.tensor.matmul(
                        sps[:, i, :],
                        lhsT=KT_sb[:, h, c * 128:(c + 1) * 128],
                        rhs=qblk, start=True, stop=True,
                    )
                if prev is not None:
                    for i in range(2):
                        c = 2 * g + i
                        nc.tensor.matmul(
                            ops, lhsT=V_sb[:, c, ph * 128:(ph + 1) * 128],
                            rhs=pPT[:, c, :], start=(c == 0), stop=(c == SC - 1),
                            skip_group_check=True,
                        )
                nc.scalar.activation(
                    PT[:, 2 * g:2 * (g + 1), :], sps, EXP, scale=SCALE
                )
            if prev is not None:
                finalize(pj, ph, pPT, ops)
            return PT



        def outproj(j):
            OT = OT_tiles[j]
            for m in range(4):
                for n in range(2):
                    yps = ypsum.tile([128, BLK], F32, name="yps", tag="yps")
                    for h in range(H):
                        nc.tensor.matmul(
                            yps, lhsT=OT[:, h, m * 128:(m + 1) * 128],
                            rhs=Wo_sb[:, h, n * BLK:(n + 1) * BLK],
                            start=(h == 0), stop=(h == H - 1),
                        )
                    y_sb = p2.tile([128, BLK], F32, name="y_sb", tag="y", bufs=3)
                    nc.vector.tensor_add(
                        y_sb, yps, bo_bc[:, n * BLK:(n + 1) * BLK]
                    )
                    r0 = j * BLK + m * 128
                    nc.sync.dma_start(
                        out=out[r0:r0 + 128, n * BLK:(n + 1) * BLK], in_=y_sb
                    )

        seq = [(j, h) for j in range(NBLK) for h in range(H)]
        prev = None
        for j, h in seq:
            if h == 0:
                OT_tiles[j] = p2.tile(
                    [128, H, BLK], BF16, name="OT", tag="OT", bufs=2
                )
            PT = head_iter((j, h), prev)
            if h == 1 and j > 0:
                outproj(j - 1)
            prev = (j, h, PT)

        # tail: the last head's PV matmuls trickle behind its exps, so
        # fill the drain (and the finalize latency) with outproj(3)
        # groups whose h=7 contribution is deferred until OT(3,7) lands.
        lj, lh, lPT = prev
        OT = OT_tiles[lj]

        def ygroup_partial(m, n):
            yps = ypsum.tile([128, BLK], F32, name="yps", tag="yps")
            for h in range(H - 1):
                nc.tensor.matmul(
                    yps, lhsT=OT[:, h, m * 128:(m + 1) * 128],
                    rhs=Wo_sb[:, h, n * BLK:(n + 1) * BLK],
                    start=(h == 0), stop=False, skip_group_check=True,
                )
            return yps

        def ygroup_close(m, n, yps):
            nc.tensor.matmul(
                yps, lhsT=OT[:, H - 1, m * 128:(m + 1) * 128],
                rhs=Wo_sb[:, H - 1, n * BLK:(n + 1) * BLK],
                start=False, stop=True, skip_group_check=True,
            )
            y_sb = p2.tile([128, BLK], F32, name="y_sb", tag="y", bufs=3)
            nc.vector.tensor_add(y_sb, yps, bo_bc[:, n * BLK:(n + 1) * BLK])
            r0 = lj * BLK + m * 128
            nc.sync.dma_start(
                out=out[r0:r0 + 128, n * BLK:(n + 1) * BLK], in_=y_sb
            )

        groups = [(m, n) for m in range(4) for n in range(2)]
        openg = [(groups[0], ygroup_partial(*groups[0]))]
        ops = opsum.tile([128, BLK], F32, name="ops", tag="ops")
        for c in range(SC):
            nc.tensor.matmul(
                ops, lhsT=V_sb[:, c, lh * 128:(lh + 1) * 128],
                rhs=lPT[:, c, :], start=(c == 0), stop=(c == SC - 1),
                skip_group_check=True,
            )
        finalize(lj, lh, lPT, ops)
        openg.append((groups[1], ygroup_partial(*groups[1])))
        for t in range(8):
            (m, n), yps = openg.pop(0)
            ygroup_close(m, n, yps)
            if t + 2 < 8:
                g = groups[t + 2]
                openg.append((g, ygroup_partial(*g)))
    persist.release()


_CACHE = {}


def _build():
    if "nc" in _CACHE:
        return _CACHE["nc"]
    nc = bacc.Bacc(
        "TRN2", target_bir_lowering=False, debug=False,
        enable_asserts=False, num_devices=B,
    )
    aps = {
        "query": nc.dram_tensor("query", [S, D], BF16, kind="ExternalInput").ap(),
        "key_value": nc.dram_tensor("key_value", [S, D], BF16, kind="ExternalInput").ap(),
        "Wq": nc.dram_tensor("Wq", [D, H * DK], BF16, kind="ExternalInput").ap(),
        "Wk": nc.dram_tensor("Wk", [D, H * DK], BF16, kind="ExternalInput").ap(),
        "Wv": nc.dram_tensor("Wv", [D, H * DV], BF16, kind="ExternalInput").ap(),
        "Wo": nc.dram_tensor("Wo", [H * DV, D], BF16, kind="ExternalInput").ap(),
        "bo": nc.dram_tensor("bo", [1, D], F32, kind="ExternalInput").ap(),
        "out": nc.dram_tensor("out", [S, D], F32, kind="ExternalOutput").ap(),
    }
    with tile.TileContext(nc) as tc:
        _emit(tc, aps)
    nc.compile()
    _CACHE["nc"] = nc
    return nc


LAST_RESULT = None


def kernel(query, key_value, Wq, Wk, Wv, Wo, bo):
    global LAST_RESULT
    import ml_dtypes

    BF = ml_dtypes.bfloat16
    nc = _build()
    query = np.ascontiguousarray(np.asarray(query, dtype=np.float32).astype(BF))
    key_value = np.ascontiguousarray(
        np.asarray(key_value, dtype=np.float32).astype(BF)
    )
    shared = {
        "Wq": np.ascontiguousarray(np.asarray(Wq, dtype=np.float32).astype(BF)),
        "Wk": np.ascontiguousarray(np.asarray(Wk, dtype=np.float32).astype(BF)),
        "Wv": np.ascontiguousarray(np.asarray(Wv, dtype=np.float32).astype(BF)),
        "Wo": np.ascontiguousarray(np.asarray(Wo, dtype=np.float32).astype(BF)),
        "bo": np.ascontiguousarray(np.asarray(bo, dtype=np.float32)).reshape(1, D),
    }
    in_maps = [
        {"query": query[i], "key_value": key_value[i], **shared} for i in range(B)
    ]
    res = run_bass_kernel_spmd(
        nc, in_maps, core_ids=list(range(B)),
        trace=bool(int(os.environ.get("KERNEL_TRACE", "0"))),
    )
    LAST_RESULT = res
    return np.stack([r["out"] for r in res.results]).astype(np.float32)


if __name__ == "__main__":
    rng = np.random.default_rng(0)
    inputs = {
        "query": rng.standard_normal((B, S, D), dtype=np.float32),
        "key_value": rng.standard_normal((B, S, D), dtype=np.float32),
        "Wq": (rng.random((D, H * DK), dtype=np.float32) - 0.5) / 16.0,
        "Wk": (rng.random((D, H * DK), dtype=np.float32) - 0.5) / 16.0,
        "Wv": (rng.random((D, H * DV), dtype=np.float32) - 0.5) / 16.0,
        "Wo": (rng.random((H * DV, D), dtype=np.float32) - 0.5) / 16.0,
        "bo": (rng.random(D, dtype=np.float32) - 0.5) / 16.0,
    }
    y = kernel(**inputs)
    print("kernel out", y.shape, y.dtype, float(np.abs(y).max()))
